# revision 16
# baseline (speedup 1.0000x reference)
"""Trainium2 Bass kernel for nn_AlignmentLoss (topk_masking).

Computation (per batch b):
    avg_attn = mean over (H, Lq) of cross_attn_weights[b]        # [Lc]
    idx      = top5(avg_attn)                                    # [5]
    top_ctx  = context_emb[b, idx]                               # [5, D]
    q_vec    = mean over Lq of question_emb[b]                   # [D]
    sim_k    = cos(q_vec, top_ctx[k])  (eps-clamped norms)
    loss_b   = mean_k (1 - sim_k)
loss = mean_b loss_b

Sharding: pure data-parallel over B=8 across 8 NeuronCores (1 batch/core).
Each core reads its 32 MB attention slab (the dominant traffic), reduces it
on the TensorEngine with a ones-vector matmul, finds top-5 with the DVE
max/max_index ops, gathers 5 context rows with an indirect DMA (so the
16 MB context slab is never streamed), and emits a single scalar.  The
host averages the 8 scalars (the "all-reduce mean").
"""

from contextlib import ExitStack

import numpy as np

import concourse.bass as bass
import concourse.tile as tile
from concourse import bacc, mybir
from concourse.bass_utils import run_bass_kernel_spmd

B, H, Lq, Lc, D = 8, 16, 128, 4096, 1024
R = H * Lq               # 2048 rows to reduce per batch
KT = R // 128            # 16 k-tiles
NCH = Lc // 512          # 8 psum chunks of 512
NCORES = 8
EPS = 1e-8
F32 = mybir.dt.float32
BF16 = mybir.dt.bfloat16

_CACHE: dict = {}


def emit_body(nc, tc, es, attn, q, ctx, out, rep, mode="full"):
    """One full per-core computation; writes loss scalar to out[0, rep]."""
    sfx = f"_{rep}"
    cpool = es.enter_context(tc.tile_pool(name="const" + sfx, bufs=1))
    wpool = es.enter_context(tc.tile_pool(name="w" + sfx, bufs=6))
    spool = es.enter_context(tc.tile_pool(name="small" + sfx, bufs=1))

    ones = cpool.tile([128, 1], F32)
    nc.vector.memset(ones[:], 1.0)

    if mode == "full":
        # ---- q path: qhat = q_sum / max(||q_sum||, eps) (scale-invariant) ----
        qt = spool.tile([128, D], F32)
        nc.sync.dma_start(qt[:], q[:, :])
        qs = spool.tile([1, D], F32)
        with tc.tile_pool(name="psq_pool" + sfx, bufs=1, space="PSUM") as pq:
            psq = pq.tile([1, D], F32)
            nc.tensor.matmul(out=psq[:, 0:512], lhsT=ones[:], rhs=qt[:, 0:512],
                             start=True, stop=True)
            nc.tensor.matmul(out=psq[:, 512:1024], lhsT=ones[:],
                             rhs=qt[:, 512:1024], start=True, stop=True)
            nc.vector.tensor_copy(qs[:], psq[:])
        qscr = spool.tile([1, D], F32)
        qsq = spool.tile([1, 1], F32)
        nc.scalar.activation(qscr[:], qs[:],
                             mybir.ActivationFunctionType.Square,
                             accum_out=qsq[:])
        qn = spool.tile([1, 1], F32)
        nc.scalar.sqrt(qn[:], qsq[:])
        nc.vector.tensor_scalar_max(qn[:], qn[:], EPS)
        qinv = spool.tile([1, 1], F32)
        nc.vector.reciprocal(qinv[:], qn[:])
        qhat = spool.tile([1, D], F32)
        nc.vector.tensor_scalar_mul(qhat[:], qs[:], qinv[:, 0:1])

    # ---- main loop: column sums of attn (hi/lo bf16 split) into psum ----
    # attn is [KT, 2, 128, Lc] bf16: slab k holds the k-th row-tile's bf16
    # hi part (s=0) and bf16 residual lo part (s=1); hi+lo sums reproduce
    # the fp32 column sums to ~2^-18 relative while streaming the PE at
    # bf16 rate (fp32 matmul is 4x slower).
    ones_bf = cpool.tile([128, 1], BF16)
    nc.vector.memset(ones_bf[:], 1.0)
    avg = spool.tile([1, Lc], F32)
    with tc.tile_pool(name="pacc_pool" + sfx, bufs=1, space="PSUM") as pa:
        pacc = pa.tile([1, Lc], F32)
        engines = [nc.sync, nc.scalar, nc.gpsimd]
        for k in range(KT):
            wt = wpool.tile([128, 2 * Lc], BF16, tag="w")
            engines[k % 3].dma_start(
                wt[:].rearrange("p (s c) -> p s c", s=2),
                attn[k].rearrange("s p c -> p s c"))
            for s in range(2):
                for n in range(NCH):
                    nc.tensor.matmul(
                        out=pacc[:, n * 512:(n + 1) * 512],
                        lhsT=ones_bf[:],
                        rhs=wt[:, s * Lc + n * 512:s * Lc + (n + 1) * 512],
                        start=(k == 0 and s == 0),
                        stop=(k == KT - 1 and s == 1))

        # ---- assemble avg in SBUF ----
        for n in range(NCH):
            sl = slice(n * 512, (n + 1) * 512)
            if n % 2 == 0:
                nc.vector.tensor_copy(avg[:, sl], pacc[:, sl])
            else:
                nc.scalar.copy(avg[:, sl], pacc[:, sl])

    if mode == "attn":
        nc.sync.dma_start(out[0:1, rep:rep + 1], avg[0:1, 0:1])
        return

    # ---- top-5 ----
    vals8 = spool.tile([1, 8], F32)
    idx8 = spool.tile([1, 8], mybir.dt.uint32)
    nc.vector.max(vals8[:], avg[:])
    nc.vector.max_index(idx8[:], vals8[:], avg[:])
    if mode == "topk":
        nc.sync.dma_start(out[0:1, rep:rep + 1], vals8[0:1, 0:1])
        return

    # scatter the first 5 indices across partitions for the gather
    idx5 = spool.tile([5, 1], mybir.dt.uint32)
    nc.sync.dma_start(idx5[:, 0:1], idx8[0:1, 0:5])

    # ---- gather 5 context rows, cosine ----
    ctx5 = spool.tile([5, D], F32)
    nc.gpsimd.indirect_dma_start(
        out=ctx5[:], out_offset=None, in_=ctx[:, :],
        in_offset=bass.IndirectOffsetOnAxis(ap=idx5[:, 0:1], axis=0))
    qb5 = spool.tile([5, D], F32)
    nc.gpsimd.partition_broadcast(qb5[:], qhat[0:1, :])
    scr1 = spool.tile([5, D], F32)
    dots = spool.tile([5, 1], F32)
    nc.vector.tensor_tensor(out=scr1[:], in0=ctx5[:], in1=qb5[:],
                            op=mybir.AluOpType.mult)
    nc.vector.reduce_sum(dots[:], scr1[:], axis=mybir.AxisListType.X)
    scr2 = spool.tile([5, D], F32)
    csq = spool.tile([5, 1], F32)
    nc.scalar.activation(scr2[:], ctx5[:], mybir.ActivationFunctionType.Square,
                         accum_out=csq[:])
    cn = spool.tile([5, 1], F32)
    nc.scalar.sqrt(cn[:], csq[:])
    nc.vector.tensor_scalar_max(cn[:], cn[:], EPS)
    cinv = spool.tile([5, 1], F32)
    nc.vector.reciprocal(cinv[:], cn[:])
    sim5 = spool.tile([5, 1], F32)
    nc.vector.tensor_tensor(out=sim5[:], in0=dots[:], in1=cinv[:],
                            op=mybir.AluOpType.mult)

    # loss = 1 - mean(sim5): ones[0:5].T @ sim5 -> [1,1], then *(-1/5)+1
    lossT = spool.tile([1, 1], F32)
    with tc.tile_pool(name="psl_pool" + sfx, bufs=1, space="PSUM") as pl:
        psl = pl.tile([1, 1], F32)
        nc.tensor.matmul(out=psl[:], lhsT=ones[0:5, 0:1], rhs=sim5[0:5, 0:1],
                         start=True, stop=True)
        nc.scalar.activation(lossT[:], psl[:],
                             mybir.ActivationFunctionType.Copy,
                             bias=1.0, scale=-1.0 / 5.0)
    nc.sync.dma_start(out[0:1, rep:rep + 1], lossT[:])


def build_nc(reps=1, mode="full"):
    nc = bacc.Bacc("TRN2", target_bir_lowering=False, debug=False)
    attn = nc.dram_tensor("attn", [KT, 2, 128, Lc], BF16,
                          kind="ExternalInput").ap()
    q = nc.dram_tensor("q", [Lq, D], F32, kind="ExternalInput").ap()
    ctx = nc.dram_tensor("ctx", [Lc, D], F32, kind="ExternalInput").ap()
    out = nc.dram_tensor("out", [1, reps], F32, kind="ExternalOutput").ap()

    with tile.TileContext(nc) as tc:
        for rep in range(reps):
            with ExitStack() as es:
                emit_body(nc, tc, es, attn, q, ctx, out, rep, mode=mode)

    nc.compile()
    return nc


def get_nc(reps=1, mode="full"):
    key = ("nc", reps, mode)
    if key not in _CACHE:
        _CACHE[key] = build_nc(reps, mode)
    return _CACHE[key]


def make_in_maps(question_emb, context_emb, cross_attn_weights):
    import ml_dtypes

    bf16 = ml_dtypes.bfloat16
    qe = np.ascontiguousarray(np.asarray(question_emb, dtype=np.float32))
    ce = np.ascontiguousarray(np.asarray(context_emb, dtype=np.float32))
    caw = np.asarray(cross_attn_weights, dtype=np.float32)
    assert qe.shape == (B, Lq, D) and ce.shape == (B, Lc, D)
    assert caw.shape == (B, H, Lq, Lc)
    # hi/lo bf16 split of the attention weights (lossless to ~2^-18 rel)
    flat = caw.reshape(B, KT, 128, Lc)
    hi = flat.astype(bf16)
    lo = (flat - hi.astype(np.float32)).astype(bf16)
    attn_hl = np.stack([hi, lo], axis=2)  # [B, KT, 2, 128, Lc]
    return [
        {
            "attn": attn_hl[b],
            "q": qe[b],
            "ctx": ce[b],
        }
        for b in range(B)
    ]


def kernel(question_emb, context_emb, cross_attn_weights, **_unused):
    nc = get_nc()
    in_maps = make_in_maps(question_emb, context_emb, cross_attn_weights)
    res = run_bass_kernel_spmd(nc, in_maps, core_ids=list(range(NCORES)))
    losses = [res.results[c]["out"][0, 0] for c in range(NCORES)]
    return np.float32(np.mean(losses))


# revision 19
# speedup vs baseline: 1.2379x; 1.2379x over previous
"""Trainium2 Bass kernel for nn_AlignmentLoss (topk_masking).

Computation (per batch b):
    avg_attn = mean over (H, Lq) of cross_attn_weights[b]        # [Lc]
    idx      = top5(avg_attn)                                    # [5]
    top_ctx  = context_emb[b, idx]                               # [5, D]
    q_vec    = mean over Lq of question_emb[b]                   # [D]
    sim_k    = cos(q_vec, top_ctx[k])  (eps-clamped norms)
    loss_b   = mean_k (1 - sim_k)
loss = mean_b loss_b

Sharding: pure data-parallel over B=8 across 8 NeuronCores (1 batch/core).
Each core reads its 32 MB attention slab (the dominant traffic), reduces it
on the TensorEngine with a ones-vector matmul, finds top-5 with the DVE
max/max_index ops, gathers 5 context rows with an indirect DMA (so the
16 MB context slab is never streamed), and emits a single scalar.  The
host averages the 8 scalars (the "all-reduce mean").
"""

from contextlib import ExitStack

import numpy as np

import concourse.bass as bass
import concourse.tile as tile
from concourse import bacc, mybir
from concourse.bass_utils import run_bass_kernel_spmd

B, H, Lq, Lc, D = 8, 16, 128, 4096, 1024
R = H * Lq               # 2048 rows to reduce per batch
KT = R // 128            # 16 k-tiles
NCH = Lc // 512          # 8 psum chunks of 512
NCORES = 8
EPS = 1e-8
F32 = mybir.dt.float32
BF16 = mybir.dt.bfloat16
F8 = mybir.dt.float8e4

_CACHE: dict = {}


def emit_body(nc, tc, es, attn_h, attn_l, q, ctx, out, rep, mode="full"):
    """One full per-core computation; writes loss scalar to out[0, rep]."""
    sfx = f"_{rep}"
    cpool = es.enter_context(tc.tile_pool(name="const" + sfx, bufs=1))
    wpool = es.enter_context(tc.tile_pool(name="w" + sfx, bufs=4))
    spool = es.enter_context(tc.tile_pool(name="small" + sfx, bufs=1))

    ones = cpool.tile([128, 1], F32)
    nc.vector.memset(ones[:], 1.0)

    if mode == "full":
        # ---- q path: qhat = q_sum / max(||q_sum||, eps) (scale-invariant) ----
        qt = spool.tile([128, D], F32)
        nc.sync.dma_start(qt[:], q[:, :])
        qs = spool.tile([1, D], F32)
        with tc.tile_pool(name="psq_pool" + sfx, bufs=1, space="PSUM") as pq:
            psq = pq.tile([1, D], F32)
            nc.tensor.matmul(out=psq[:, 0:512], lhsT=ones[:], rhs=qt[:, 0:512],
                             start=True, stop=True)
            nc.tensor.matmul(out=psq[:, 512:1024], lhsT=ones[:],
                             rhs=qt[:, 512:1024], start=True, stop=True)
            nc.vector.tensor_copy(qs[:], psq[:])
        qscr = spool.tile([1, D], F32)
        qsq = spool.tile([1, 1], F32)
        nc.scalar.activation(qscr[:], qs[:],
                             mybir.ActivationFunctionType.Square,
                             accum_out=qsq[:])
        qn = spool.tile([1, 1], F32)
        nc.scalar.sqrt(qn[:], qsq[:])
        nc.vector.tensor_scalar_max(qn[:], qn[:], EPS)
        qinv = spool.tile([1, 1], F32)
        nc.vector.reciprocal(qinv[:], qn[:])
        qhat = spool.tile([1, D], F32)
        nc.vector.tensor_scalar_mul(qhat[:], qs[:], qinv[:, 0:1])

    # ---- main loop: column sums of attn (hi/lo bf16 split) into psum ----
    # attn is [KT, 2, 128, Lc] bf16: slab k holds the k-th row-tile's bf16
    # hi part (s=0) and bf16 residual lo part (s=1); hi+lo sums reproduce
    # the fp32 column sums to ~2^-18 relative while streaming the PE at
    # bf16 rate (fp32 matmul is 4x slower).
    # hi stream: bf16, summed with a ones vector.  lo stream: residuals
    # pre-scaled by 2**13 on the host and stored fp8e4m3; the stationary
    # vector is 2**-13 (exact in bf16), so the PE applies the descale for
    # free while accumulating into the same PSUM group.
    ones_bf = cpool.tile([128, 1], BF16)
    nc.vector.memset(ones_bf[:], 1.0)
    ones_lo = cpool.tile([128, 1], BF16)
    nc.vector.memset(ones_lo[:], 2.0 ** -13)
    avg = spool.tile([1, Lc], F32)
    TPG = 2  # k-slabs per DMA pair
    with tc.tile_pool(name="pacc_pool" + sfx, bufs=1, space="PSUM") as pa:
        pacc = pa.tile([1, Lc], F32)
        for g in range(KT // TPG):
            wh = wpool.tile([128, TPG * Lc], BF16, tag="wh")
            wl = wpool.tile([128, TPG * Lc], F8, tag="wl")
            e1 = nc.sync if (g % 2 == 0) else nc.scalar
            e2 = nc.scalar if (g % 2 == 0) else nc.sync
            e1.dma_start(
                wh[:].rearrange("p (t c) -> p t c", t=TPG),
                attn_h[g * TPG:(g + 1) * TPG].rearrange("t p c -> p t c"))
            e2.dma_start(
                wl[:].rearrange("p (t c) -> p t c", t=TPG),
                attn_l[g * TPG:(g + 1) * TPG].rearrange("t p c -> p t c"))
            for t in range(TPG):
                for n in range(NCH):
                    sl = slice(t * Lc + n * 512, t * Lc + (n + 1) * 512)
                    nc.tensor.matmul(
                        out=pacc[:, n * 512:(n + 1) * 512],
                        lhsT=ones_bf[:], rhs=wh[:, sl],
                        start=(g == 0 and t == 0), stop=False)
                    nc.tensor.matmul(
                        out=pacc[:, n * 512:(n + 1) * 512],
                        lhsT=ones_lo[:], rhs=wl[:, sl],
                        start=False,
                        stop=(g == KT // TPG - 1 and t == TPG - 1))

        # ---- assemble avg in SBUF ----
        for n in range(NCH):
            sl = slice(n * 512, (n + 1) * 512)
            if n % 2 == 0:
                nc.vector.tensor_copy(avg[:, sl], pacc[:, sl])
            else:
                nc.scalar.copy(avg[:, sl], pacc[:, sl])

    if mode == "attn":
        nc.sync.dma_start(out[0:1, rep:rep + 1], avg[0:1, 0:1])
        return

    # ---- top-5 ----
    vals8 = spool.tile([1, 8], F32)
    idx8 = spool.tile([1, 8], mybir.dt.uint32)
    nc.vector.max(vals8[:], avg[:])
    nc.vector.max_index(idx8[:], vals8[:], avg[:])
    if mode == "topk":
        nc.sync.dma_start(out[0:1, rep:rep + 1], vals8[0:1, 0:1])
        return

    # scatter the first 5 indices across partitions for the gather
    idx5 = spool.tile([5, 1], mybir.dt.uint32)
    nc.sync.dma_start(idx5[:, 0:1], idx8[0:1, 0:5])

    # ---- gather 5 context rows, cosine ----
    ctx5 = spool.tile([5, D], F32)
    nc.gpsimd.indirect_dma_start(
        out=ctx5[:], out_offset=None, in_=ctx[:, :],
        in_offset=bass.IndirectOffsetOnAxis(ap=idx5[:, 0:1], axis=0))
    qb5 = spool.tile([5, D], F32)
    nc.gpsimd.partition_broadcast(qb5[:], qhat[0:1, :])
    scr1 = spool.tile([5, D], F32)
    dots = spool.tile([5, 1], F32)
    nc.vector.tensor_tensor(out=scr1[:], in0=ctx5[:], in1=qb5[:],
                            op=mybir.AluOpType.mult)
    nc.vector.reduce_sum(dots[:], scr1[:], axis=mybir.AxisListType.X)
    scr2 = spool.tile([5, D], F32)
    csq = spool.tile([5, 1], F32)
    nc.scalar.activation(scr2[:], ctx5[:], mybir.ActivationFunctionType.Square,
                         accum_out=csq[:])
    cn = spool.tile([5, 1], F32)
    nc.scalar.sqrt(cn[:], csq[:])
    nc.vector.tensor_scalar_max(cn[:], cn[:], EPS)
    cinv = spool.tile([5, 1], F32)
    nc.vector.reciprocal(cinv[:], cn[:])
    sim5 = spool.tile([5, 1], F32)
    nc.vector.tensor_tensor(out=sim5[:], in0=dots[:], in1=cinv[:],
                            op=mybir.AluOpType.mult)

    # loss = 1 - mean(sim5): ones[0:5].T @ sim5 -> [1,1], then *(-1/5)+1
    lossT = spool.tile([1, 1], F32)
    with tc.tile_pool(name="psl_pool" + sfx, bufs=1, space="PSUM") as pl:
        psl = pl.tile([1, 1], F32)
        nc.tensor.matmul(out=psl[:], lhsT=ones[0:5, 0:1], rhs=sim5[0:5, 0:1],
                         start=True, stop=True)
        nc.scalar.activation(lossT[:], psl[:],
                             mybir.ActivationFunctionType.Copy,
                             bias=1.0, scale=-1.0 / 5.0)
    nc.sync.dma_start(out[0:1, rep:rep + 1], lossT[:])


def build_nc(reps=1, mode="full"):
    nc = bacc.Bacc("TRN2", target_bir_lowering=False, debug=False)
    attn_h = nc.dram_tensor("attn_h", [KT, 128, Lc], BF16,
                            kind="ExternalInput").ap()
    attn_l = nc.dram_tensor("attn_l", [KT, 128, Lc], F8,
                            kind="ExternalInput").ap()
    q = nc.dram_tensor("q", [Lq, D], F32, kind="ExternalInput").ap()
    ctx = nc.dram_tensor("ctx", [Lc, D], F32, kind="ExternalInput").ap()
    out = nc.dram_tensor("out", [1, reps], F32, kind="ExternalOutput").ap()

    with tile.TileContext(nc) as tc:
        for rep in range(reps):
            with ExitStack() as es:
                emit_body(nc, tc, es, attn_h, attn_l, q, ctx, out, rep,
                          mode=mode)

    nc.compile()
    return nc


def get_nc(reps=1, mode="full"):
    key = ("nc", reps, mode)
    if key not in _CACHE:
        _CACHE[key] = build_nc(reps, mode)
    return _CACHE[key]


def make_in_maps(question_emb, context_emb, cross_attn_weights):
    import ml_dtypes

    bf16 = ml_dtypes.bfloat16
    qe = np.ascontiguousarray(np.asarray(question_emb, dtype=np.float32))
    ce = np.ascontiguousarray(np.asarray(context_emb, dtype=np.float32))
    caw = np.asarray(cross_attn_weights, dtype=np.float32)
    assert qe.shape == (B, Lq, D) and ce.shape == (B, Lc, D)
    assert caw.shape == (B, H, Lq, Lc)
    # hi (bf16) + scaled-residual lo (fp8e4m3) split of the attention
    # weights: top-k selection error stays ~1e-3 on sums of ~1024 while
    # the stream shrinks from 32 MB to 24 MB per core.
    f8 = ml_dtypes.float8_e4m3
    flat = caw.reshape(B, KT, 128, Lc)
    hi = flat.astype(bf16)
    lo8 = ((flat - hi.astype(np.float32)) * 8192.0).astype(f8)
    return [
        {
            "attn_h": hi[b],
            "attn_l": lo8[b],
            "q": qe[b],
            "ctx": ce[b],
        }
        for b in range(B)
    ]


def kernel(question_emb, context_emb, cross_attn_weights, **_unused):
    nc = get_nc()
    in_maps = make_in_maps(question_emb, context_emb, cross_attn_weights)
    res = run_bass_kernel_spmd(nc, in_maps, core_ids=list(range(NCORES)))
    losses = [res.results[c]["out"][0, 0] for c in range(NCORES)]
    return np.float32(np.mean(losses))


# revision 20
# speedup vs baseline: 1.4032x; 1.1336x over previous
"""Trainium2 Bass kernel for nn_AlignmentLoss (topk_masking).

Computation (per batch b):
    avg_attn = mean over (H, Lq) of cross_attn_weights[b]        # [Lc]
    idx      = top5(avg_attn)                                    # [5]
    top_ctx  = context_emb[b, idx]                               # [5, D]
    q_vec    = mean over Lq of question_emb[b]                   # [D]
    sim_k    = cos(q_vec, top_ctx[k])  (eps-clamped norms)
    loss_b   = mean_k (1 - sim_k)
loss = mean_b loss_b

Sharding: pure data-parallel over B=8 across 8 NeuronCores (1 batch/core).
Each core reads its 32 MB attention slab (the dominant traffic), reduces it
on the TensorEngine with a ones-vector matmul, finds top-5 with the DVE
max/max_index ops, gathers 5 context rows with an indirect DMA (so the
16 MB context slab is never streamed), and emits a single scalar.  The
host averages the 8 scalars (the "all-reduce mean").
"""

from contextlib import ExitStack

import numpy as np

import concourse.bass as bass
import concourse.tile as tile
from concourse import bacc, mybir
from concourse.bass_utils import run_bass_kernel_spmd

B, H, Lq, Lc, D = 8, 16, 128, 4096, 1024
R = H * Lq               # 2048 rows to reduce per batch
KT = R // 128            # 16 k-tiles
NCH = Lc // 512          # 8 psum chunks of 512
NCORES = 8
EPS = 1e-8
F32 = mybir.dt.float32
BF16 = mybir.dt.bfloat16
F8 = mybir.dt.float8e4

_CACHE: dict = {}


def emit_body(nc, tc, es, attn_h, attn_l, q, ctx, out, rep, mode="full"):
    """One full per-core computation; writes loss scalar to out[0, rep]."""
    sfx = f"_{rep}"
    cpool = es.enter_context(tc.tile_pool(name="const" + sfx, bufs=1))
    wpool = es.enter_context(tc.tile_pool(name="w" + sfx, bufs=4))
    spool = es.enter_context(tc.tile_pool(name="small" + sfx, bufs=1))

    ones = cpool.tile([128, 1], F32)
    nc.vector.memset(ones[:], 1.0)

    if mode == "full":
        # ---- q path: qhat = q_sum / max(||q_sum||, eps) (scale-invariant) ----
        qt = spool.tile([128, D], F32)
        nc.sync.dma_start(qt[:], q[:, :])
        qs = spool.tile([1, D], F32)
        with tc.tile_pool(name="psq_pool" + sfx, bufs=1, space="PSUM") as pq:
            psq = pq.tile([1, D], F32)
            nc.tensor.matmul(out=psq[:, 0:512], lhsT=ones[:], rhs=qt[:, 0:512],
                             start=True, stop=True)
            nc.tensor.matmul(out=psq[:, 512:1024], lhsT=ones[:],
                             rhs=qt[:, 512:1024], start=True, stop=True)
            nc.vector.tensor_copy(qs[:], psq[:])
        qscr = spool.tile([1, D], F32)
        qsq = spool.tile([1, 1], F32)
        nc.scalar.activation(qscr[:], qs[:],
                             mybir.ActivationFunctionType.Square,
                             accum_out=qsq[:])
        qn = spool.tile([1, 1], F32)
        nc.scalar.sqrt(qn[:], qsq[:])
        nc.vector.tensor_scalar_max(qn[:], qn[:], EPS)
        qinv = spool.tile([1, 1], F32)
        nc.vector.reciprocal(qinv[:], qn[:])
        qhat = spool.tile([1, D], F32)
        nc.vector.tensor_scalar_mul(qhat[:], qs[:], qinv[:, 0:1])

    # ---- main loop: column sums of attn (hi/lo bf16 split) into psum ----
    # attn is [KT, 2, 128, Lc] bf16: slab k holds the k-th row-tile's bf16
    # hi part (s=0) and bf16 residual lo part (s=1); hi+lo sums reproduce
    # the fp32 column sums to ~2^-18 relative while streaming the PE at
    # bf16 rate (fp32 matmul is 4x slower).
    # hi stream: bf16, summed with a ones vector.  lo stream: residuals
    # pre-scaled by 2**13 on the host and stored fp8e4m3; the stationary
    # vector is 2**-13 (exact in bf16), so the PE applies the descale for
    # free while accumulating into the same PSUM group.
    ones_bf = cpool.tile([128, 1], BF16)
    nc.vector.memset(ones_bf[:], 1.0)
    ones_lo = cpool.tile([128, 1], BF16)
    nc.vector.memset(ones_lo[:], 2.0 ** -13)
    avg = spool.tile([1, Lc], F32)
    TPG = 2  # k-slabs per DMA pair
    with tc.tile_pool(name="pacc_pool" + sfx, bufs=1, space="PSUM") as pa:
        pacc = pa.tile([1, Lc], F32)
        for g in range(KT // TPG):
            wh = wpool.tile([128, TPG * Lc], BF16, tag="wh")
            wl = wpool.tile([128, TPG * Lc], F8, tag="wl")
            e1 = nc.sync if (g % 2 == 0) else nc.scalar
            e2 = nc.scalar if (g % 2 == 0) else nc.sync
            # balance the two HWDGE rings: one hi slab per ring, lo
            # alternating -> 1.5 MB per ring per group
            e1.dma_start(wh[:, 0:Lc], attn_h[g * TPG])
            e2.dma_start(wh[:, Lc:2 * Lc], attn_h[g * TPG + 1])
            e1.dma_start(
                wl[:].rearrange("p (t c) -> p t c", t=TPG),
                attn_l[g * TPG:(g + 1) * TPG].rearrange("t p c -> p t c"))
            for t in range(TPG):
                for n in range(NCH):
                    sl = slice(t * Lc + n * 512, t * Lc + (n + 1) * 512)
                    nc.tensor.matmul(
                        out=pacc[:, n * 512:(n + 1) * 512],
                        lhsT=ones_bf[:], rhs=wh[:, sl],
                        start=(g == 0 and t == 0), stop=False)
                    nc.tensor.matmul(
                        out=pacc[:, n * 512:(n + 1) * 512],
                        lhsT=ones_lo[:], rhs=wl[:, sl],
                        start=False,
                        stop=(g == KT // TPG - 1 and t == TPG - 1))

        # ---- assemble avg in SBUF ----
        for n in range(NCH):
            sl = slice(n * 512, (n + 1) * 512)
            if n % 2 == 0:
                nc.vector.tensor_copy(avg[:, sl], pacc[:, sl])
            else:
                nc.scalar.copy(avg[:, sl], pacc[:, sl])

    if mode == "attn":
        nc.sync.dma_start(out[0:1, rep:rep + 1], avg[0:1, 0:1])
        return

    # ---- top-5 ----
    vals8 = spool.tile([1, 8], F32)
    idx8 = spool.tile([1, 8], mybir.dt.uint32)
    nc.vector.max(vals8[:], avg[:])
    nc.vector.max_index(idx8[:], vals8[:], avg[:])
    if mode == "topk":
        nc.sync.dma_start(out[0:1, rep:rep + 1], vals8[0:1, 0:1])
        return

    # scatter the first 5 indices across partitions for the gather
    idx5 = spool.tile([5, 1], mybir.dt.uint32)
    nc.sync.dma_start(idx5[:, 0:1], idx8[0:1, 0:5])

    # ---- gather 5 context rows, cosine ----
    ctx5 = spool.tile([5, D], F32)
    nc.gpsimd.indirect_dma_start(
        out=ctx5[:], out_offset=None, in_=ctx[:, :],
        in_offset=bass.IndirectOffsetOnAxis(ap=idx5[:, 0:1], axis=0))
    qb5 = spool.tile([5, D], F32)
    nc.gpsimd.partition_broadcast(qb5[:], qhat[0:1, :])
    scr1 = spool.tile([5, D], F32)
    dots = spool.tile([5, 1], F32)
    nc.vector.tensor_tensor(out=scr1[:], in0=ctx5[:], in1=qb5[:],
                            op=mybir.AluOpType.mult)
    nc.vector.reduce_sum(dots[:], scr1[:], axis=mybir.AxisListType.X)
    scr2 = spool.tile([5, D], F32)
    csq = spool.tile([5, 1], F32)
    nc.scalar.activation(scr2[:], ctx5[:], mybir.ActivationFunctionType.Square,
                         accum_out=csq[:])
    cn = spool.tile([5, 1], F32)
    nc.scalar.sqrt(cn[:], csq[:])
    nc.vector.tensor_scalar_max(cn[:], cn[:], EPS)
    cinv = spool.tile([5, 1], F32)
    nc.vector.reciprocal(cinv[:], cn[:])
    sim5 = spool.tile([5, 1], F32)
    nc.vector.tensor_tensor(out=sim5[:], in0=dots[:], in1=cinv[:],
                            op=mybir.AluOpType.mult)

    # loss = 1 - mean(sim5): ones[0:5].T @ sim5 -> [1,1], then *(-1/5)+1
    lossT = spool.tile([1, 1], F32)
    with tc.tile_pool(name="psl_pool" + sfx, bufs=1, space="PSUM") as pl:
        psl = pl.tile([1, 1], F32)
        nc.tensor.matmul(out=psl[:], lhsT=ones[0:5, 0:1], rhs=sim5[0:5, 0:1],
                         start=True, stop=True)
        nc.scalar.activation(lossT[:], psl[:],
                             mybir.ActivationFunctionType.Copy,
                             bias=1.0, scale=-1.0 / 5.0)
    nc.sync.dma_start(out[0:1, rep:rep + 1], lossT[:])


def build_nc(reps=1, mode="full"):
    nc = bacc.Bacc("TRN2", target_bir_lowering=False, debug=False)
    attn_h = nc.dram_tensor("attn_h", [KT, 128, Lc], BF16,
                            kind="ExternalInput").ap()
    attn_l = nc.dram_tensor("attn_l", [KT, 128, Lc], F8,
                            kind="ExternalInput").ap()
    q = nc.dram_tensor("q", [Lq, D], F32, kind="ExternalInput").ap()
    ctx = nc.dram_tensor("ctx", [Lc, D], F32, kind="ExternalInput").ap()
    out = nc.dram_tensor("out", [1, reps], F32, kind="ExternalOutput").ap()

    with tile.TileContext(nc) as tc:
        for rep in range(reps):
            with ExitStack() as es:
                emit_body(nc, tc, es, attn_h, attn_l, q, ctx, out, rep,
                          mode=mode)

    nc.compile()
    return nc


def get_nc(reps=1, mode="full"):
    key = ("nc", reps, mode)
    if key not in _CACHE:
        _CACHE[key] = build_nc(reps, mode)
    return _CACHE[key]


def make_in_maps(question_emb, context_emb, cross_attn_weights):
    import ml_dtypes

    bf16 = ml_dtypes.bfloat16
    qe = np.ascontiguousarray(np.asarray(question_emb, dtype=np.float32))
    ce = np.ascontiguousarray(np.asarray(context_emb, dtype=np.float32))
    caw = np.asarray(cross_attn_weights, dtype=np.float32)
    assert qe.shape == (B, Lq, D) and ce.shape == (B, Lc, D)
    assert caw.shape == (B, H, Lq, Lc)
    # hi (bf16) + scaled-residual lo (fp8e4m3) split of the attention
    # weights: top-k selection error stays ~1e-3 on sums of ~1024 while
    # the stream shrinks from 32 MB to 24 MB per core.
    f8 = ml_dtypes.float8_e4m3
    flat = caw.reshape(B, KT, 128, Lc)
    hi = flat.astype(bf16)
    lo8 = ((flat - hi.astype(np.float32)) * 8192.0).astype(f8)
    return [
        {
            "attn_h": hi[b],
            "attn_l": lo8[b],
            "q": qe[b],
            "ctx": ce[b],
        }
        for b in range(B)
    ]


def kernel(question_emb, context_emb, cross_attn_weights, **_unused):
    nc = get_nc()
    in_maps = make_in_maps(question_emb, context_emb, cross_attn_weights)
    res = run_bass_kernel_spmd(nc, in_maps, core_ids=list(range(NCORES)))
    losses = [res.results[c]["out"][0, 0] for c in range(NCORES)]
    return np.float32(np.mean(losses))
